# revision 33
# baseline (speedup 1.0000x reference)
"""Multi-head attention (B=2, S=2048, D=1024, H=16, dk=dv=64) on 8 trn2 cores.

Sharding: (batch, head-quad) -> core.  Core i handles batch i//4 and the 4
heads [4*(i%4), 4*(i%4)+4).  Each core computes its partial output
context_h @ W_O[h-slice] summed over its 4 heads; the host sums the 4
bf16 partials per batch in f32 (the "all-reduce" of the row-sharded output
projection).

The main loop is ACT(exp)-bound: 80 score tasks x ~1us of exp.  The
schedule therefore starts the exp stream as early as possible (first
score task after ~16 matmuls) and keeps ACT saturated: projection halves
are emitted lazily right before the first score task that needs them,
consumes trail produces by a bounded backlog, and causal trimming cuts
every diagonal task (scores matmul, exp ACTIVATE via a 3-D AP over both
heads' valid ranges, A@V matmul, and a 128-wide triangle-only mask).

PSUM discipline (in-order engine queues make ring-allocation order
load-bearing): one uniform transient ring of 2x[128,1024] (tag "sc")
carries ALL matmul outputs that are freed immediately by ACT or a DVE
copy -- scores, Q/K projection halves, V projection, output projection.
The 4x[128,512] ctx ring holds ONLY the A@V pair accumulators (two pairs
in flight), so no transient allocation can ever ring-wait on a held ctx
slot (deadlock-free by construction).

Inputs are host-packed as [128, NB, NCH, BLK] so each 512-query block of
x^T is a single DMA descriptor (descriptor issue on the Sync queue costs
~0.7us each); xv streams through a 2-deep [128, NCH, BLK] rotating pool.
"""

import os
import numpy as np
import ml_dtypes

import concourse.bacc as bacc
import concourse.tile as tile
import concourse.mybir as mybir
import concourse.bass_utils as bass_utils
from concourse.bass import ds

B, S, D, H, DK = 2, 2048, 1024, 16, 64
N_CORES = 8
HPC = 4            # heads per core
NCH = 8            # d-model chunks of 128
NB = 4             # query blocks of 512
BLK = 512
NT = 16            # s tiles of 128
VW = DK + 1        # V columns per head incl. ones column

DT = mybir.dt.bfloat16
NP_DT = ml_dtypes.bfloat16
F32 = mybir.dt.float32

TRACE = False      # set True (or BASS_TRACE=1) to capture an NTFF profile
LAST_RESULTS = None

_CACHED_NC = None


def _build_program():
    nc = bacc.Bacc("TRN2", target_bir_lowering=False, debug=False,
                   enable_asserts=False, num_devices=N_CORES)

    xq_d = nc.dram_tensor("xq_t", [128, NB, NCH, BLK], DT, kind="ExternalInput")
    xk_d = nc.dram_tensor("xk_t", [128, NB, NCH, BLK], DT, kind="ExternalInput")
    xv_d = nc.dram_tensor("xv_t", [128, NB, NCH, BLK], DT, kind="ExternalInput")
    wq_d = nc.dram_tensor("wq", [128, 2, NCH, 128], DT, kind="ExternalInput")
    wk_d = nc.dram_tensor("wk", [128, 2, NCH, 128], DT, kind="ExternalInput")
    wv_d = nc.dram_tensor("wv", [128, NCH, HPC * DK], DT, kind="ExternalInput")
    wo_d = nc.dram_tensor("wo", [128, 2, D], DT, kind="ExternalInput")
    mask_d = nc.dram_tensor("mask01", [128, 2, 128], DT, kind="ExternalInput")
    ident_d = nc.dram_tensor("ident", [128, 128], F32, kind="ExternalInput")
    out_d = nc.dram_tensor("out_partial", [S, D], DT, kind="ExternalOutput")

    with tile.TileContext(nc) as tc:
        _body(tc, xq_d, xk_d, xv_d, wq_d, wk_d, wv_d, wo_d, mask_d, ident_d,
              out_d)
    nc.compile()
    return nc


def _body(tc, xq_d, xk_d, xv_d, wq_d, wk_d, wv_d, wo_d, mask_d, ident_d,
          out_d):
    nc = tc.nc

    with (
        tc.tile_pool(name="consts", bufs=1) as consts,
        tc.tile_pool(name="persist", bufs=1) as persist,
        tc.tile_pool(name="small", bufs=3) as small,
    ):
        # ---- constants ----
        wq_sb = consts.tile([128, 2, NCH, 128], DT)
        wk_sb = consts.tile([128, 2, NCH, 128], DT)
        wv_sb = consts.tile([128, NCH, HPC * DK], DT)
        wo_sb = consts.tile([128, 2, D], DT)
        mask_sb = consts.tile([128, 2, 128], DT)
        ident_sb = consts.tile([128, 128], F32)

        # ---- persistent activations ----
        qt_sb = persist.tile([128, 2, S], DT)        # Q^T, pair-major
        kt_sb = persist.tile([128, 2, S], DT)        # K^T
        v_sb = persist.tile([128, NT, HPC * VW], DT)  # V + ones cols
        ctxt_sb = persist.tile([128, 2, S], DT)      # context^T

        ones_sb = persist.tile([1, DK], F32)    # for the 1/denom broadcast
        nc.vector.memset(ones_sb[:], 1.0)
        for hh in range(HPC):
            nc.vector.memset(v_sb[:, :, hh * VW + DK: hh * VW + DK + 1], 1.0)

        with (
            tc.tile_pool(name="xqk", bufs=1) as xqk_pool,
            tc.tile_pool(name="xv", bufs=2) as xv_pool,
            tc.tile_pool(name="pt", bufs=16) as pt_pool,
            tc.tile_pool(name="osb", bufs=3) as out_pool,
            tc.tile_pool(name="psum_sc", bufs=3, space="PSUM") as sc_pool,
            tc.tile_pool(name="psum_ctx", bufs=2, space="PSUM") as ctx_pool,
        ):
            st = dict(sc_pool=sc_pool, ctx_pool=ctx_pool,
                      pt_pool=pt_pool, out_pool=out_pool, small=small,
                      qt=qt_sb, kt=kt_sb, v=v_sb, ctxt=ctxt_sb,
                      mask=mask_sb, wo=wo_sb, out_d=out_d, nc=nc,
                      ones=ones_sb,
                      EXP=mybir.ActivationFunctionType.Exp,
                      MUL=mybir.AluOpType.mult)
            xq_sb = xqk_pool.tile([128, NB, NCH, BLK], DT)
            xk_sb = xqk_pool.tile([128, NB, NCH, BLK], DT)

            # ---- DMA issue order = need order ----
            nc.sync.dma_start(wq_sb[:, 0], wq_d[:, 0])
            for c4 in range(0, NCH, 2):
                nc.sync.dma_start(xq_sb[:, 0, c4:c4 + 2],
                                  xq_d[:, 0, c4:c4 + 2])
            nc.sync.dma_start(wk_sb[:, 0], wk_d[:, 0])
            for c4 in range(0, NCH, 2):
                nc.sync.dma_start(xk_sb[:, 0, c4:c4 + 2],
                                  xk_d[:, 0, c4:c4 + 2])
            nc.sync.dma_start(wq_sb[:, 1], wq_d[:, 1])
            nc.sync.dma_start(wk_sb[:, 1], wk_d[:, 1])
            nc.sync.dma_start(mask_sb[:], mask_d[:])
            nc.sync.dma_start(ident_sb[:], ident_d[:])
            xv_bufs = {}

            def xv_fetch(b):
                t = xv_pool.tile([128, NCH, BLK], DT, name="xvb", tag="xvb")
                nc.sync.dma_start(t[:], xv_d[:, b])
                xv_bufs[b] = t

            xv_fetch(0)
            nc.sync.dma_start(wv_sb[:], wv_d[:])
            nc.sync.dma_start(xq_sb[:, 1:NB], xq_d[:, 1:NB])
            nc.sync.dma_start(xk_sb[:, 1:NB], xk_d[:, 1:NB])
            nc.sync.dma_start(wo_sb[:], wo_d[:])

            # ---- weave state ----
            queue = []      # produced-but-unconsumed (blk, hp, skt, pt, q0)
            ctx_maps = {}   # (blk, hp) -> {h: psum tile}
            norms_done = [0]
            v_emitted = [0]
            consumed = [0]
            op_ready = []   # (tile, min_consumed)
            proj_emitted = set()

            def transient():
                return sc_pool.tile([128, 1024], F32, name="sc", tag="sc")

            def proj_half(dst, w_sb, blk0, nblk, p):
                x_sb = xq_sb if dst is qt_sb else xk_sb
                w = BLK * nblk
                ps = transient()
                for c in range(NCH):
                    nc.tensor.matmul(
                        ps[:, 0:w],
                        lhsT=w_sb[:, p, c, :],
                        rhs=x_sb[:, blk0:blk0 + nblk, c, :],
                        start=(c == 0), stop=(c == NCH - 1))
                nc.vector.tensor_copy(dst[:, p, ds(BLK * blk0, w)],
                                      ps[:, 0:w])

            # projection emission groups (one matmul output <= one PSUM
            # bank = 512 fp32, so groups stay single-block)
            PGRP = {0: (0, 1), 1: (1, 1), 2: (2, 1), 3: (3, 1)}

            def ensure_proj(which, blk, p):
                g = PGRP[blk]
                if (which, g, p) in proj_emitted:
                    return
                proj_emitted.add((which, g, p))
                if which == "q":
                    proj_half(qt_sb, wq_sb, g[0], g[1], p)
                else:
                    proj_half(kt_sb, wk_sb, g[0], g[1], p)

            def proj_v_block(b):
                # W_V-stationary 512-col matmuls produce V^T (8 MMs per
                # head pair instead of 32 short ones), then PE-mode
                # transposes put it in the [sk, dv] layout A@V needs.
                if b + 1 < NB and (b + 1) not in xv_bufs:
                    xv_fetch(b + 1)
                x = xv_bufs.pop(b)
                for j in range(2):
                    ps = transient()
                    for c in range(NCH):
                        nc.tensor.matmul(
                            ps[:, 0:BLK],
                            lhsT=wv_sb[:, c, ds(128 * j, 128)],
                            rhs=x[:, c, :],
                            start=(c == 0), stop=(c == NCH - 1))
                    vt = st["small"].tile([128, BLK], F32, name="vt",
                                          tag="vt")
                    nc.vector.tensor_copy(vt[:], ps[:, 0:BLK])
                    for i in range(4):
                        t = 4 * b + i
                        tp = transient()
                        nc.tensor.transpose(tp[:, 0:128],
                                            vt[:, ds(128 * i, 128)],
                                            ident_sb[:])
                        dst = v_sb[:, t, :].rearrange(
                            "p (hh e) -> p hh e",
                            hh=HPC)[:, 2 * j:2 * j + 2, 0:DK]
                        srcap = tp[:, 0:128].rearrange(
                            "p (hh e) -> p hh e", hh=2)
                        nc.vector.tensor_copy(dst, srcap)
                v_emitted[0] = 4 * b + 4

            def produce_step(blk, hp, skt):
                pt, q0 = _produce(st, blk, hp, skt)
                queue.append((blk, hp, skt, pt, q0))

            # outproj tiles released per normalize event; emitted one per
            # consume, >=2 consumes after the event, so the normalize chain
            # (DVE/gpsimd) never stalls the next A@V matmuls behind them in
            # the in-order PE queue.
            op_sched = {3: [0, 1, 2, 3], 7: [4, 5, 6, 7],
                        11: [8, 9, 10, 11], 15: [12, 13, 14, 15]}

            def do_consume():
                blk, hp, skt, pt, q0 = queue.pop(0)
                while 4 * (blk + 1) > v_emitted[0]:
                    proj_v_block(v_emitted[0] // 4)
                ctxps = ctx_maps.setdefault((blk, hp), {})
                _consume(st, blk, hp, skt, pt, q0, ctxps)
                consumed[0] += 1
                if op_ready and op_ready[0][1] <= consumed[0]:
                    _outproj_tile(st, op_ready.pop(0)[0])
                if skt == 4 * (blk + 1) - 1:  # pair complete
                    tail = (blk == NB - 1 and hp == 1)
                    for hh2 in range(2):
                        _normalize(st, blk, hp, hh2, ctxps, tail)
                        norms_done[0] += 1
                        for t in op_sched.get(norms_done[0] - 1, []):
                            op_ready.append((t, consumed[0] + 2))
                    del ctx_maps[(blk, hp)]

            # ---- the weave ----
            total = sum(4 * (b + 1) for b in range(NB)) * 2
            emitted = 0
            for blk in range(NB):
                for hp in range(2):
                    ensure_proj("q", blk, hp)
                    for skt in range(4 * (blk + 1)):
                        ensure_proj("k", skt // 4, hp)
                        produce_step(blk, hp, skt)
                        emitted += 1
                        while len(queue) > min(8, total - emitted):
                            do_consume()
            while queue:
                do_consume()
            while op_ready:
                _outproj_tile(st, op_ready.pop(0)[0])


def _produce(st, blk, hp, skt):
    """Scores matmuls + exp (+ causal triangle mask) for one task.

    The two heads of the pair sit on disjoint PE row groups (lhsT base
    partitions 0 and 64), so their back-to-back scores matmuls execute
    concurrently in the array; both heads' P^T share one [128,1024] tile
    (head h2 in columns 512*h2..512*h2+512) and one exp ACTIVATE.
    Diagonal tasks (skt >= 4*blk) only touch query columns >= 128*u."""
    nc = st["nc"]
    u = skt - 4 * blk
    q0 = 128 * u if u > 0 else 0
    sc = st["sc_pool"].tile([128, 1024], F32, name="sc", tag="sc")
    for h2 in range(2):
        nc.tensor.matmul(
            sc[:, ds(512 * h2 + q0, 512 - q0)],
            lhsT=st["kt"][ds(64 * h2, 64), hp, ds(128 * skt, 128)],
            rhs=st["qt"][ds(64 * h2, 64), hp, ds(BLK * blk + q0, BLK - q0)],
            start=True, stop=True)
    pt = st["pt_pool"].tile([128, 1024], DT, name="pt", tag="pt")
    sc3 = sc[:].rearrange("p (h q) -> p h q", h=2)[:, :, q0:BLK]
    pt3 = pt[:].rearrange("p (h q) -> p h q", h=2)[:, :, q0:BLK]
    nc.scalar.activation(pt3, sc3, st["EXP"], scale=0.125)
    if u >= 0:  # diagonal: zero the in-tile upper triangle (both heads)
        tri = pt[:].rearrange("p (h q) -> p h q", h=2)[:, :, q0:q0 + 128]
        nc.vector.tensor_tensor(tri, tri, st["mask"][:], st["MUL"])
    return pt, q0


def _consume(st, blk, hp, skt, pt, q0, ctxps):
    """A@V accumulation for one produced task (both heads of the pair)."""
    nc = st["nc"]
    last = 4 * (blk + 1) - 1
    for h2 in range(2):
        h = 2 * hp + h2
        if h not in ctxps:
            ctxps[h] = st["ctx_pool"].tile(
                [128, BLK], F32, name=f"ctx{h2}", tag="ctx")
        nc.tensor.matmul(
            ctxps[h][0:DK + 1, ds(q0, BLK - q0)],
            lhsT=st["v"][:, skt, ds(h * VW, VW)],
            rhs=pt[:, ds(512 * h2 + q0, BLK - q0)],
            start=(skt == 0), stop=(skt == last))


def _normalize(st, blk, hp, h2, ctxps, tail=False):
    """ctx rows 0..63 scaled by 1/row64 -> ctx^T bf16.

    Normally the ctx PSUM slot is released by two quick DVE copies (sums
    row + ctx rows into SBUF) so the reciprocal/broadcast/multiply run off
    the critical path and the next pair's A@V is not stalled.  For the
    last pair (tail=True) the multiply reads the ctx PSUM directly
    (nothing follows, and skipping the raw copy shortens the final
    normalize->outproj chain), and a result-unused PE outer product keeps
    the HAM clock-gate warm through the drain."""
    nc = st["nc"]
    h = 2 * hp + h2
    # custom-DVE ops read garbage from PSUM -> plain-copy the sums row to
    # SBUF first (DVE copy of [1,512] is cheap; DVE reads PSUM fine).
    sums = st["small"].tile([1, BLK], F32, name="sums", tag="sums")
    nc.vector.tensor_copy(sums[:], ctxps[h][ds(DK, 1), :])
    if tail:
        raw = ctxps[h][0:64, :]
    else:
        raw_t = st["small"].tile([64, BLK], F32, name="raw", tag="raw")
        nc.vector.tensor_copy(raw_t[:], ctxps[h][0:64, :])
        raw = raw_t[:]
    r = st["small"].tile([1, BLK], F32, name="r", tag="r")
    nc.vector.reciprocal_approx_fast(out=r[:], in_=sums[:])
    if tail:  # HAM warm-keeper (result never read; slot frees on write)
        wm = st["sc_pool"].tile([128, 1024], F32, name="sc", tag="sc")
        nc.tensor.matmul(wm[0:DK, 0:BLK], lhsT=st["ones"][:], rhs=r[:],
                         start=True, stop=True)
    bc = st["small"].tile([64, BLK], F32, name="bc", tag="bc")
    nc.gpsimd.partition_broadcast(bc[:], r[:])
    nc.vector.tensor_tensor(
        st["ctxt"][ds(64 * h2, 64), hp, ds(BLK * blk, BLK)],
        raw, bc[:], st["MUL"])


def _outproj_tile(st, t):
    nc = st["nc"]
    ob = st["out_pool"].tile([128, D], DT, name="ob", tag="ob")
    pp = st["sc_pool"].tile([128, 1024], F32, name="sc", tag="sc")
    for nb in range(2):
        for cc in range(2):
            nc.tensor.matmul(
                pp[:, ds(512 * nb, 512)],
                lhsT=st["ctxt"][:, cc, ds(128 * t, 128)],
                rhs=st["wo"][:, cc, ds(512 * nb, 512)],
                start=(cc == 0), stop=(cc == 1))
    nc.vector.tensor_copy(ob[:], pp[:])
    nc.sync.dma_start(st["out_d"][ds(128 * t, 128), :], ob[:])


def _make_mask():
    # tri[i, h, j] = 1.0 iff key-within-tile i <= query-within-group j,
    # duplicated for the two heads of a pair (shared P^T tile).
    i = np.arange(128)[:, None]
    j = np.arange(128)[None, :]
    tri = (i <= j).astype(NP_DT)
    return np.ascontiguousarray(np.stack([tri, tri], axis=1))


def _prep_core_inputs(inputs, core):
    b = core // 4
    h0 = HPC * (core % 4)
    c0, c1 = h0 * DK, (h0 + HPC) * DK
    f32 = np.float32

    def t_blocks(x):  # [S, D] -> [128, NB, NCH, BLK]
        xt = np.ascontiguousarray(np.asarray(x, f32).T)  # [D, S]
        return np.ascontiguousarray(
            xt.reshape(NCH, 128, NB, BLK).transpose(1, 2, 0, 3)
        ).astype(NP_DT)

    return {
        "xq_t": t_blocks(inputs["input_Q"][b]),
        "xk_t": t_blocks(inputs["input_K"][b]),
        "xv_t": t_blocks(inputs["input_V"][b]),
        "wq": np.ascontiguousarray(np.asarray(inputs["W_Q"], f32)[:, c0:c1].reshape(NCH, 128, 2, 128).transpose(1, 2, 0, 3)).astype(NP_DT),
        "wk": np.ascontiguousarray(np.asarray(inputs["W_K"], f32)[:, c0:c1].reshape(NCH, 128, 2, 128).transpose(1, 2, 0, 3)).astype(NP_DT),
        "wv": np.ascontiguousarray(np.asarray(inputs["W_V"], f32)[:, c0:c1].reshape(NCH, 128, HPC * DK).transpose(1, 0, 2)).astype(NP_DT),
        "wo": np.ascontiguousarray(np.asarray(inputs["W_O"], f32)[c0:c1, :].reshape(2, 128, D).transpose(1, 0, 2)).astype(NP_DT),
        "mask01": _make_mask(),
        "ident": np.eye(128, dtype=f32),
    }


def get_program():
    global _CACHED_NC
    if _CACHED_NC is None:
        _CACHED_NC = _build_program()
    return _CACHED_NC


def kernel(**inputs):
    global LAST_RESULTS
    nc = get_program()
    in_maps = [_prep_core_inputs(inputs, core) for core in range(N_CORES)]
    res = bass_utils.run_bass_kernel_spmd(
        nc, in_maps, core_ids=list(range(N_CORES)),
        trace=TRACE or bool(int(os.environ.get("BASS_TRACE", "0") or 0)))
    LAST_RESULTS = res
    out = np.zeros((B, S, D), np.float32)
    for core in range(N_CORES):
        out[core // 4] += np.asarray(
            res.results[core]["out_partial"], dtype=np.float32)
    return out


# revision 34
# speedup vs baseline: 1.0561x; 1.0561x over previous
"""Multi-head attention (B=2, S=2048, D=1024, H=16, dk=dv=64) on 8 trn2 cores.

Sharding: (batch, head-quad) -> core.  Core i handles batch i//4 and the 4
heads [4*(i%4), 4*(i%4)+4).  Each core computes its partial output
context_h @ W_O[h-slice] summed over its 4 heads; the host sums the 4
bf16 partials per batch in f32 (the "all-reduce" of the row-sharded output
projection).

The main loop is ACT(exp)-bound: 80 score tasks x ~1us of exp.  The
schedule therefore starts the exp stream as early as possible (first
score task after ~16 matmuls) and keeps ACT saturated: projection halves
are emitted lazily right before the first score task that needs them,
consumes trail produces by a bounded backlog, and causal trimming cuts
every diagonal task (scores matmul, exp ACTIVATE via a 3-D AP over both
heads' valid ranges, A@V matmul, and a 128-wide triangle-only mask).

PSUM discipline (in-order engine queues make ring-allocation order
load-bearing): one uniform transient ring of 2x[128,1024] (tag "sc")
carries ALL matmul outputs that are freed immediately by ACT or a DVE
copy -- scores, Q/K projection halves, V projection, output projection.
The 4x[128,512] ctx ring holds ONLY the A@V pair accumulators (two pairs
in flight), so no transient allocation can ever ring-wait on a held ctx
slot (deadlock-free by construction).

Inputs are host-packed as [128, NB, NCH, BLK] so each 512-query block of
x^T is a single DMA descriptor (descriptor issue on the Sync queue costs
~0.7us each); xv streams through a 2-deep [128, NCH, BLK] rotating pool.
"""

import os
import numpy as np
import ml_dtypes

import concourse.bacc as bacc
import concourse.tile as tile
import concourse.mybir as mybir
import concourse.bass_utils as bass_utils
from concourse.bass import ds

B, S, D, H, DK = 2, 2048, 1024, 16, 64
N_CORES = 8
HPC = 4            # heads per core
NCH = 8            # d-model chunks of 128
NB = 4             # query blocks of 512
BLK = 512
NT = 16            # s tiles of 128
VW = DK + 1        # V columns per head incl. ones column

DT = mybir.dt.bfloat16
NP_DT = ml_dtypes.bfloat16
F32 = mybir.dt.float32

TRACE = False      # set True (or BASS_TRACE=1) to capture an NTFF profile
LAST_RESULTS = None

_CACHED_NC = None


def _build_program():
    nc = bacc.Bacc("TRN2", target_bir_lowering=False, debug=False,
                   enable_asserts=False, num_devices=N_CORES)

    xq_d = nc.dram_tensor("xq_t", [128, NB, NCH, BLK], DT, kind="ExternalInput")
    xk_d = nc.dram_tensor("xk_t", [128, NB, NCH, BLK], DT, kind="ExternalInput")
    xv_d = nc.dram_tensor("xv_t", [128, NB, NCH, BLK], DT, kind="ExternalInput")
    wq_d = nc.dram_tensor("wq", [128, 2, NCH, 128], DT, kind="ExternalInput")
    wk_d = nc.dram_tensor("wk", [128, 2, NCH, 128], DT, kind="ExternalInput")
    wv_d = nc.dram_tensor("wv", [128, NCH, HPC * DK], DT, kind="ExternalInput")
    wo_d = nc.dram_tensor("wo", [128, 2, D], DT, kind="ExternalInput")
    mask_d = nc.dram_tensor("mask01", [128, 2, 128], DT, kind="ExternalInput")
    ident_d = nc.dram_tensor("ident", [128, 128], F32, kind="ExternalInput")
    out_d = nc.dram_tensor("out_partial", [S, D], DT, kind="ExternalOutput")

    with tile.TileContext(nc) as tc:
        _body(tc, xq_d, xk_d, xv_d, wq_d, wk_d, wv_d, wo_d, mask_d, ident_d,
              out_d)
    nc.compile()
    return nc


def _body(tc, xq_d, xk_d, xv_d, wq_d, wk_d, wv_d, wo_d, mask_d, ident_d,
          out_d):
    nc = tc.nc

    with (
        tc.tile_pool(name="consts", bufs=1) as consts,
        tc.tile_pool(name="persist", bufs=1) as persist,
        tc.tile_pool(name="small", bufs=3) as small,
    ):
        # ---- constants ----
        wq_sb = consts.tile([128, 2, NCH, 128], DT)
        wk_sb = consts.tile([128, 2, NCH, 128], DT)
        wv_sb = consts.tile([128, NCH, HPC * DK], DT)
        wo_sb = consts.tile([128, 2, D], DT)
        mask_sb = consts.tile([128, 2, 128], DT)
        ident_sb = consts.tile([128, 128], F32)

        # ---- persistent activations ----
        qt_sb = persist.tile([128, 2, S], DT)        # Q^T, pair-major
        kt_sb = persist.tile([128, 2, S], DT)        # K^T
        v_sb = persist.tile([128, NT, HPC * VW], DT)  # V + ones cols
        ctxt_sb = persist.tile([128, 2, S], DT)      # context^T

        ones_sb = persist.tile([1, DK], F32)    # for the 1/denom broadcast
        nc.vector.memset(ones_sb[:], 1.0)
        for hh in range(HPC):
            nc.vector.memset(v_sb[:, :, hh * VW + DK: hh * VW + DK + 1], 1.0)

        with (
            tc.tile_pool(name="xqk", bufs=1) as xqk_pool,
            tc.tile_pool(name="xv", bufs=3) as xv_pool,
            tc.tile_pool(name="pt", bufs=16) as pt_pool,
            tc.tile_pool(name="osb", bufs=3) as out_pool,
            tc.tile_pool(name="psum_sc", bufs=3, space="PSUM") as sc_pool,
            tc.tile_pool(name="psum_ctx", bufs=2, space="PSUM") as ctx_pool,
        ):
            st = dict(sc_pool=sc_pool, ctx_pool=ctx_pool,
                      pt_pool=pt_pool, out_pool=out_pool, small=small,
                      qt=qt_sb, kt=kt_sb, v=v_sb, ctxt=ctxt_sb,
                      mask=mask_sb, wo=wo_sb, out_d=out_d, nc=nc,
                      ones=ones_sb,
                      EXP=mybir.ActivationFunctionType.Exp,
                      MUL=mybir.AluOpType.mult)
            xq_sb = xqk_pool.tile([128, NB, NCH, BLK], DT)
            xk_sb = xqk_pool.tile([128, NB, NCH, BLK], DT)

            # ---- DMA issue order = need order ----
            nc.sync.dma_start(wq_sb[:, 0], wq_d[:, 0])
            for c4 in range(0, NCH, 2):
                nc.sync.dma_start(xq_sb[:, 0, c4:c4 + 2],
                                  xq_d[:, 0, c4:c4 + 2])
            nc.sync.dma_start(wk_sb[:, 0], wk_d[:, 0])
            for c4 in range(0, NCH, 2):
                nc.sync.dma_start(xk_sb[:, 0, c4:c4 + 2],
                                  xk_d[:, 0, c4:c4 + 2])
            nc.sync.dma_start(wq_sb[:, 1], wq_d[:, 1])
            nc.sync.dma_start(wk_sb[:, 1], wk_d[:, 1])
            nc.sync.dma_start(mask_sb[:], mask_d[:])
            nc.sync.dma_start(ident_sb[:], ident_d[:])
            xv_bufs = {}

            def xv_fetch(b):
                t = xv_pool.tile([128, NCH, BLK], DT, name="xvb", tag="xvb")
                nc.sync.dma_start(t[:], xv_d[:, b])
                xv_bufs[b] = t

            xv_fetch(0)
            nc.sync.dma_start(wv_sb[:], wv_d[:])
            nc.sync.dma_start(xq_sb[:, 1:NB], xq_d[:, 1:NB])
            nc.sync.dma_start(xk_sb[:, 1:NB], xk_d[:, 1:NB])
            nc.sync.dma_start(wo_sb[:], wo_d[:])

            # ---- weave state ----
            queue = []      # produced-but-unconsumed (blk, hp, skt, pt, q0)
            ctx_maps = {}   # (blk, hp) -> {h: psum tile}
            norms_done = [0]
            v_emitted = [0]
            consumed = [0]
            op_ready = []   # (tile, min_consumed)
            proj_emitted = set()

            def transient():
                return sc_pool.tile([128, 1024], F32, name="sc", tag="sc")

            def proj_half(dst, w_sb, blk0, nblk, p):
                x_sb = xq_sb if dst is qt_sb else xk_sb
                w = BLK * nblk
                ps = transient()
                for c in range(NCH):
                    nc.tensor.matmul(
                        ps[:, 0:w],
                        lhsT=w_sb[:, p, c, :],
                        rhs=x_sb[:, blk0:blk0 + nblk, c, :],
                        start=(c == 0), stop=(c == NCH - 1))
                nc.vector.tensor_copy(dst[:, p, ds(BLK * blk0, w)],
                                      ps[:, 0:w])

            # projection emission groups (one matmul output <= one PSUM
            # bank = 512 fp32, so groups stay single-block)
            PGRP = {0: (0, 1), 1: (1, 1), 2: (2, 1), 3: (3, 1)}

            def ensure_proj(which, blk, p):
                g = PGRP[blk]
                if (which, g, p) in proj_emitted:
                    return
                proj_emitted.add((which, g, p))
                if which == "q":
                    proj_half(qt_sb, wq_sb, g[0], g[1], p)
                else:
                    proj_half(kt_sb, wk_sb, g[0], g[1], p)

            def proj_v_block(b):
                for nb in range(b + 1, min(NB, b + 3)):
                    if nb not in xv_bufs:
                        xv_fetch(nb)
                x = xv_bufs.pop(b)
                for t in range(4 * b, 4 * b + 4):
                    ps = transient()
                    for c in range(NCH):
                        nc.tensor.matmul(
                            ps[:, 0:HPC * DK],
                            lhsT=x[:, c, ds(128 * (t - 4 * b), 128)],
                            rhs=wv_sb[:, c, :],
                            start=(c == 0), stop=(c == NCH - 1))
                    dst = v_sb[:, t, :].rearrange(
                        "p (hh e) -> p hh e", hh=HPC)[:, :, 0:DK]
                    srcap = ps[:, 0:HPC * DK].rearrange(
                        "p (hh e) -> p hh e", hh=HPC)
                    nc.vector.tensor_copy(dst, srcap)
                v_emitted[0] = 4 * b + 4

            def produce_step(blk, hp, skt):
                pt, q0 = _produce(st, blk, hp, skt)
                queue.append((blk, hp, skt, pt, q0))

            # outproj tiles released per normalize event; emitted one per
            # consume, >=2 consumes after the event, so the normalize chain
            # (DVE/gpsimd) never stalls the next A@V matmuls behind them in
            # the in-order PE queue.
            op_sched = {3: [0, 1, 2, 3], 7: [4, 5, 6, 7],
                        11: [8, 9, 10, 11], 15: [12, 13, 14, 15]}

            def do_consume():
                blk, hp, skt, pt, q0 = queue.pop(0)
                while 4 * (blk + 1) > v_emitted[0]:
                    proj_v_block(v_emitted[0] // 4)
                ctxps = ctx_maps.setdefault((blk, hp), {})
                _consume(st, blk, hp, skt, pt, q0, ctxps)
                consumed[0] += 1
                if op_ready and op_ready[0][1] <= consumed[0]:
                    _outproj_tile(st, op_ready.pop(0)[0])
                if skt == 4 * (blk + 1) - 1:  # pair complete
                    tail = (blk == NB - 1 and hp == 1)
                    for hh2 in range(2):
                        _normalize(st, blk, hp, hh2, ctxps, tail)
                        norms_done[0] += 1
                        for t in op_sched.get(norms_done[0] - 1, []):
                            op_ready.append((t, consumed[0] + 2))
                    del ctx_maps[(blk, hp)]

            # ---- the weave ----
            total = sum(4 * (b + 1) for b in range(NB)) * 2
            emitted = 0
            for blk in range(NB):
                for hp in range(2):
                    ensure_proj("q", blk, hp)
                    for skt in range(4 * (blk + 1)):
                        ensure_proj("k", skt // 4, hp)
                        produce_step(blk, hp, skt)
                        emitted += 1
                        # early V-projection filler keeps the PE dense in
                        # the trimmed-diag stretches of blocks 1-2
                        if (blk, hp, skt) == (1, 1, 3) and v_emitted[0] == 8:
                            proj_v_block(2)
                        if (blk, hp, skt) == (2, 0, 5) and v_emitted[0] == 12:
                            proj_v_block(3)
                        while len(queue) > min(8, total - emitted):
                            do_consume()
            while queue:
                do_consume()
            while op_ready:
                _outproj_tile(st, op_ready.pop(0)[0])


def _produce(st, blk, hp, skt):
    """Scores matmuls + exp (+ causal triangle mask) for one task.

    The two heads of the pair sit on disjoint PE row groups (lhsT base
    partitions 0 and 64), so their back-to-back scores matmuls execute
    concurrently in the array; both heads' P^T share one [128,1024] tile
    (head h2 in columns 512*h2..512*h2+512) and one exp ACTIVATE.
    Diagonal tasks (skt >= 4*blk) only touch query columns >= 128*u."""
    nc = st["nc"]
    u = skt - 4 * blk
    q0 = 128 * u if u > 0 else 0
    sc = st["sc_pool"].tile([128, 1024], F32, name="sc", tag="sc")
    for h2 in range(2):
        nc.tensor.matmul(
            sc[:, ds(512 * h2 + q0, 512 - q0)],
            lhsT=st["kt"][ds(64 * h2, 64), hp, ds(128 * skt, 128)],
            rhs=st["qt"][ds(64 * h2, 64), hp, ds(BLK * blk + q0, BLK - q0)],
            start=True, stop=True)
    pt = st["pt_pool"].tile([128, 1024], DT, name="pt", tag="pt")
    sc3 = sc[:].rearrange("p (h q) -> p h q", h=2)[:, :, q0:BLK]
    pt3 = pt[:].rearrange("p (h q) -> p h q", h=2)[:, :, q0:BLK]
    nc.scalar.activation(pt3, sc3, st["EXP"], scale=0.125)
    if u >= 0:  # diagonal: zero the in-tile upper triangle (both heads)
        tri = pt[:].rearrange("p (h q) -> p h q", h=2)[:, :, q0:q0 + 128]
        nc.vector.tensor_tensor(tri, tri, st["mask"][:], st["MUL"])
    return pt, q0


def _consume(st, blk, hp, skt, pt, q0, ctxps):
    """A@V accumulation for one produced task (both heads of the pair)."""
    nc = st["nc"]
    last = 4 * (blk + 1) - 1
    for h2 in range(2):
        h = 2 * hp + h2
        if h not in ctxps:
            ctxps[h] = st["ctx_pool"].tile(
                [128, BLK], F32, name=f"ctx{h2}", tag="ctx")
        nc.tensor.matmul(
            ctxps[h][0:DK + 1, ds(q0, BLK - q0)],
            lhsT=st["v"][:, skt, ds(h * VW, VW)],
            rhs=pt[:, ds(512 * h2 + q0, BLK - q0)],
            start=(skt == 0), stop=(skt == last))


def _normalize(st, blk, hp, h2, ctxps, tail=False):
    """ctx rows 0..63 scaled by 1/row64 -> ctx^T bf16.

    Normally the ctx PSUM slot is released by two quick DVE copies (sums
    row + ctx rows into SBUF) so the reciprocal/broadcast/multiply run off
    the critical path and the next pair's A@V is not stalled.  For the
    last pair (tail=True) the multiply reads the ctx PSUM directly
    (nothing follows, and skipping the raw copy shortens the final
    normalize->outproj chain), and a result-unused PE outer product keeps
    the HAM clock-gate warm through the drain."""
    nc = st["nc"]
    h = 2 * hp + h2
    # custom-DVE ops read garbage from PSUM -> plain-copy the sums row to
    # SBUF first (DVE copy of [1,512] is cheap; DVE reads PSUM fine).
    sums = st["small"].tile([1, BLK], F32, name="sums", tag="sums")
    nc.vector.tensor_copy(sums[:], ctxps[h][ds(DK, 1), :])
    if tail:
        raw = ctxps[h][0:64, :]
    else:
        raw_t = st["small"].tile([64, BLK], F32, name="raw", tag="raw")
        nc.vector.tensor_copy(raw_t[:], ctxps[h][0:64, :])
        raw = raw_t[:]
    r = st["small"].tile([1, BLK], F32, name="r", tag="r")
    nc.vector.reciprocal_approx_fast(out=r[:], in_=sums[:])
    if tail:  # HAM warm-keeper (result never read; slot frees on write)
        wm = st["sc_pool"].tile([128, 1024], F32, name="sc", tag="sc")
        nc.tensor.matmul(wm[0:DK, 0:BLK], lhsT=st["ones"][:], rhs=r[:],
                         start=True, stop=True)
    bc = st["small"].tile([64, BLK], F32, name="bc", tag="bc")
    nc.gpsimd.partition_broadcast(bc[:], r[:])
    nc.vector.tensor_tensor(
        st["ctxt"][ds(64 * h2, 64), hp, ds(BLK * blk, BLK)],
        raw, bc[:], st["MUL"])


def _outproj_tile(st, t):
    nc = st["nc"]
    ob = st["out_pool"].tile([128, D], DT, name="ob", tag="ob")
    pp = st["sc_pool"].tile([128, 1024], F32, name="sc", tag="sc")
    for nb in range(2):
        for cc in range(2):
            nc.tensor.matmul(
                pp[:, ds(512 * nb, 512)],
                lhsT=st["ctxt"][:, cc, ds(128 * t, 128)],
                rhs=st["wo"][:, cc, ds(512 * nb, 512)],
                start=(cc == 0), stop=(cc == 1))
    nc.vector.tensor_copy(ob[:], pp[:])
    nc.sync.dma_start(st["out_d"][ds(128 * t, 128), :], ob[:])


def _make_mask():
    # tri[i, h, j] = 1.0 iff key-within-tile i <= query-within-group j,
    # duplicated for the two heads of a pair (shared P^T tile).
    i = np.arange(128)[:, None]
    j = np.arange(128)[None, :]
    tri = (i <= j).astype(NP_DT)
    return np.ascontiguousarray(np.stack([tri, tri], axis=1))


def _prep_core_inputs(inputs, core):
    b = core // 4
    h0 = HPC * (core % 4)
    c0, c1 = h0 * DK, (h0 + HPC) * DK
    f32 = np.float32

    def t_blocks(x):  # [S, D] -> [128, NB, NCH, BLK]
        xt = np.ascontiguousarray(np.asarray(x, f32).T)  # [D, S]
        return np.ascontiguousarray(
            xt.reshape(NCH, 128, NB, BLK).transpose(1, 2, 0, 3)
        ).astype(NP_DT)

    return {
        "xq_t": t_blocks(inputs["input_Q"][b]),
        "xk_t": t_blocks(inputs["input_K"][b]),
        "xv_t": t_blocks(inputs["input_V"][b]),
        "wq": np.ascontiguousarray(np.asarray(inputs["W_Q"], f32)[:, c0:c1].reshape(NCH, 128, 2, 128).transpose(1, 2, 0, 3)).astype(NP_DT),
        "wk": np.ascontiguousarray(np.asarray(inputs["W_K"], f32)[:, c0:c1].reshape(NCH, 128, 2, 128).transpose(1, 2, 0, 3)).astype(NP_DT),
        "wv": np.ascontiguousarray(np.asarray(inputs["W_V"], f32)[:, c0:c1].reshape(NCH, 128, HPC * DK).transpose(1, 0, 2)).astype(NP_DT),
        "wo": np.ascontiguousarray(np.asarray(inputs["W_O"], f32)[c0:c1, :].reshape(2, 128, D).transpose(1, 0, 2)).astype(NP_DT),
        "mask01": _make_mask(),
        "ident": np.eye(128, dtype=f32),
    }


def get_program():
    global _CACHED_NC
    if _CACHED_NC is None:
        _CACHED_NC = _build_program()
    return _CACHED_NC


def kernel(**inputs):
    global LAST_RESULTS
    nc = get_program()
    in_maps = [_prep_core_inputs(inputs, core) for core in range(N_CORES)]
    res = bass_utils.run_bass_kernel_spmd(
        nc, in_maps, core_ids=list(range(N_CORES)),
        trace=TRACE or bool(int(os.environ.get("BASS_TRACE", "0") or 0)))
    LAST_RESULTS = res
    out = np.zeros((B, S, D), np.float32)
    for core in range(N_CORES):
        out[core // 4] += np.asarray(
            res.results[core]["out_partial"], dtype=np.float32)
    return out


# revision 35
# speedup vs baseline: 1.0656x; 1.0090x over previous
"""Multi-head attention (B=2, S=2048, D=1024, H=16, dk=dv=64) on 8 trn2 cores.

Sharding: (batch, head-quad) -> core.  Core i handles batch i//4 and the 4
heads [4*(i%4), 4*(i%4)+4).  Each core computes its partial output
context_h @ W_O[h-slice] summed over its 4 heads; the host sums the 4
bf16 partials per batch in f32 (the "all-reduce" of the row-sharded output
projection).

The main loop is ACT(exp)-bound: 80 score tasks x ~1us of exp.  The
schedule therefore starts the exp stream as early as possible (first
score task after ~16 matmuls) and keeps ACT saturated: projection halves
are emitted lazily right before the first score task that needs them,
consumes trail produces by a bounded backlog, and causal trimming cuts
every diagonal task (scores matmul, exp ACTIVATE via a 3-D AP over both
heads' valid ranges, A@V matmul, and a 128-wide triangle-only mask).

PSUM discipline (in-order engine queues make ring-allocation order
load-bearing): one uniform transient ring of 2x[128,1024] (tag "sc")
carries ALL matmul outputs that are freed immediately by ACT or a DVE
copy -- scores, Q/K projection halves, V projection, output projection.
The 4x[128,512] ctx ring holds ONLY the A@V pair accumulators (two pairs
in flight), so no transient allocation can ever ring-wait on a held ctx
slot (deadlock-free by construction).

Inputs are host-packed as [128, NB, NCH, BLK] so each 512-query block of
x^T is a single DMA descriptor (descriptor issue on the Sync queue costs
~0.7us each); xv streams through a 2-deep [128, NCH, BLK] rotating pool.
"""

import os
import numpy as np
import ml_dtypes

import concourse.bacc as bacc
import concourse.tile as tile
import concourse.mybir as mybir
import concourse.bass_utils as bass_utils
from concourse.bass import ds

B, S, D, H, DK = 2, 2048, 1024, 16, 64
N_CORES = 8
HPC = 4            # heads per core
NCH = 8            # d-model chunks of 128
NB = 4             # query blocks of 512
BLK = 512
NT = 16            # s tiles of 128
VW = DK + 1        # V columns per head incl. ones column

DT = mybir.dt.bfloat16
NP_DT = ml_dtypes.bfloat16
F32 = mybir.dt.float32

TRACE = False      # set True (or BASS_TRACE=1) to capture an NTFF profile
LAST_RESULTS = None

_CACHED_NC = None


def _build_program():
    nc = bacc.Bacc("TRN2", target_bir_lowering=False, debug=False,
                   enable_asserts=False, num_devices=N_CORES)

    xq_d = nc.dram_tensor("xq_t", [128, NB, NCH, BLK], DT, kind="ExternalInput")
    xk_d = nc.dram_tensor("xk_t", [128, NB, NCH, BLK], DT, kind="ExternalInput")
    xv_d = nc.dram_tensor("xv_t", [128, NB, NCH, BLK], DT, kind="ExternalInput")
    wq_d = nc.dram_tensor("wq", [128, 2, NCH, 128], DT, kind="ExternalInput")
    wk_d = nc.dram_tensor("wk", [128, 2, NCH, 128], DT, kind="ExternalInput")
    wv_d = nc.dram_tensor("wv", [128, NCH, HPC * DK], DT, kind="ExternalInput")
    wo_d = nc.dram_tensor("wo", [128, 2, D], DT, kind="ExternalInput")
    mask_d = nc.dram_tensor("mask01", [128, 2, 128], DT, kind="ExternalInput")
    ident_d = nc.dram_tensor("ident", [128, 128], F32, kind="ExternalInput")
    out_d = nc.dram_tensor("out_partial", [S, D], DT, kind="ExternalOutput")

    with tile.TileContext(nc) as tc:
        _body(tc, xq_d, xk_d, xv_d, wq_d, wk_d, wv_d, wo_d, mask_d, ident_d,
              out_d)
    nc.compile()
    return nc


def _body(tc, xq_d, xk_d, xv_d, wq_d, wk_d, wv_d, wo_d, mask_d, ident_d,
          out_d):
    nc = tc.nc

    with (
        tc.tile_pool(name="consts", bufs=1) as consts,
        tc.tile_pool(name="persist", bufs=1) as persist,
        tc.tile_pool(name="small", bufs=3) as small,
    ):
        # ---- constants ----
        wq_sb = consts.tile([128, 2, NCH, 128], DT)
        wk_sb = consts.tile([128, 2, NCH, 128], DT)
        wv_sb = consts.tile([128, NCH, HPC * DK], DT)
        wo_sb = consts.tile([128, 2, D], DT)
        mask_sb = consts.tile([128, 2, 128], DT)
        ident_sb = consts.tile([128, 128], F32)

        # ---- persistent activations ----
        qt_sb = persist.tile([128, 2, S], DT)        # Q^T, pair-major
        kt_sb = persist.tile([128, 2, S], DT)        # K^T
        v_sb = persist.tile([128, NT, HPC * VW], DT)  # V + ones cols
        ctxt_sb = persist.tile([128, 2, S], DT)      # context^T

        ones_sb = persist.tile([1, DK], F32)    # for the 1/denom broadcast
        nc.vector.memset(ones_sb[:], 1.0)
        for hh in range(HPC):
            nc.vector.memset(v_sb[:, :, hh * VW + DK: hh * VW + DK + 1], 1.0)

        with (
            tc.tile_pool(name="xqk", bufs=1) as xqk_pool,
            tc.tile_pool(name="xv", bufs=3) as xv_pool,
            tc.tile_pool(name="pt", bufs=16) as pt_pool,
            tc.tile_pool(name="osb", bufs=3) as out_pool,
            tc.tile_pool(name="psum_sc", bufs=3, space="PSUM") as sc_pool,
            tc.tile_pool(name="psum_ctx", bufs=2, space="PSUM") as ctx_pool,
        ):
            st = dict(sc_pool=sc_pool, ctx_pool=ctx_pool,
                      pt_pool=pt_pool, out_pool=out_pool, small=small,
                      qt=qt_sb, kt=kt_sb, v=v_sb, ctxt=ctxt_sb,
                      mask=mask_sb, wo=wo_sb, out_d=out_d, nc=nc,
                      ones=ones_sb,
                      EXP=mybir.ActivationFunctionType.Exp,
                      MUL=mybir.AluOpType.mult)
            xq_sb = xqk_pool.tile([128, NB, NCH, BLK], DT)
            xk_sb = xqk_pool.tile([128, NB, NCH, BLK], DT)

            # ---- DMA issue order = need order ----
            nc.sync.dma_start(wq_sb[:, 0], wq_d[:, 0])
            for c4 in range(0, NCH, 4):
                nc.sync.dma_start(xq_sb[:, 0, c4:c4 + 4],
                                  xq_d[:, 0, c4:c4 + 4])
            nc.sync.dma_start(wk_sb[:, 0], wk_d[:, 0])
            for c4 in range(0, NCH, 4):
                nc.sync.dma_start(xk_sb[:, 0, c4:c4 + 4],
                                  xk_d[:, 0, c4:c4 + 4])
            nc.sync.dma_start(wq_sb[:, 1], wq_d[:, 1])
            nc.sync.dma_start(wk_sb[:, 1], wk_d[:, 1])
            nc.sync.dma_start(mask_sb[:], mask_d[:])
            nc.sync.dma_start(ident_sb[:], ident_d[:])
            xv_bufs = {}

            def xv_fetch(b):
                t = xv_pool.tile([128, NCH, BLK], DT, name="xvb", tag="xvb")
                nc.sync.dma_start(t[:], xv_d[:, b])
                xv_bufs[b] = t

            # per-block input slices in need order: block 1 feeds the
            # (1,*) projection halves long before blocks 2/3 are touched
            nc.sync.dma_start(xq_sb[:, 1], xq_d[:, 1])
            nc.sync.dma_start(xk_sb[:, 1], xk_d[:, 1])
            xv_fetch(0)
            nc.sync.dma_start(wv_sb[:], wv_d[:])
            nc.sync.dma_start(xq_sb[:, 2], xq_d[:, 2])
            nc.sync.dma_start(xk_sb[:, 2], xk_d[:, 2])
            nc.sync.dma_start(xq_sb[:, 3], xq_d[:, 3])
            nc.sync.dma_start(xk_sb[:, 3], xk_d[:, 3])
            nc.sync.dma_start(wo_sb[:], wo_d[:])

            # ---- weave state ----
            queue = []      # produced-but-unconsumed (blk, hp, skt, pt, q0)
            ctx_maps = {}   # (blk, hp) -> {h: psum tile}
            norms_done = [0]
            v_emitted = [0]
            consumed = [0]
            op_ready = []   # (tile, min_consumed)
            proj_emitted = set()

            def transient():
                return sc_pool.tile([128, 1024], F32, name="sc", tag="sc")

            def proj_half(dst, w_sb, blk0, nblk, p):
                x_sb = xq_sb if dst is qt_sb else xk_sb
                w = BLK * nblk
                ps = transient()
                for c in range(NCH):
                    nc.tensor.matmul(
                        ps[:, 0:w],
                        lhsT=w_sb[:, p, c, :],
                        rhs=x_sb[:, blk0:blk0 + nblk, c, :],
                        start=(c == 0), stop=(c == NCH - 1))
                nc.vector.tensor_copy(dst[:, p, ds(BLK * blk0, w)],
                                      ps[:, 0:w])

            # projection emission groups (one matmul output <= one PSUM
            # bank = 512 fp32, so groups stay single-block)
            PGRP = {0: (0, 1), 1: (1, 1), 2: (2, 1), 3: (3, 1)}

            def ensure_proj(which, blk, p):
                g = PGRP[blk]
                if (which, g, p) in proj_emitted:
                    return
                proj_emitted.add((which, g, p))
                if which == "q":
                    proj_half(qt_sb, wq_sb, g[0], g[1], p)
                else:
                    proj_half(kt_sb, wk_sb, g[0], g[1], p)

            def proj_v_block(b):
                for nb in range(b + 1, min(NB, b + 3)):
                    if nb not in xv_bufs:
                        xv_fetch(nb)
                x = xv_bufs.pop(b)
                for t in range(4 * b, 4 * b + 4):
                    ps = transient()
                    for c in range(NCH):
                        nc.tensor.matmul(
                            ps[:, 0:HPC * DK],
                            lhsT=x[:, c, ds(128 * (t - 4 * b), 128)],
                            rhs=wv_sb[:, c, :],
                            start=(c == 0), stop=(c == NCH - 1))
                    dst = v_sb[:, t, :].rearrange(
                        "p (hh e) -> p hh e", hh=HPC)[:, :, 0:DK]
                    srcap = ps[:, 0:HPC * DK].rearrange(
                        "p (hh e) -> p hh e", hh=HPC)
                    nc.vector.tensor_copy(dst, srcap)
                v_emitted[0] = 4 * b + 4

            def produce_step(blk, hp, skt):
                pt, q0 = _produce(st, blk, hp, skt)
                queue.append((blk, hp, skt, pt, q0))

            # outproj tiles released per normalize event; emitted one per
            # consume, >=2 consumes after the event, so the normalize chain
            # (DVE/gpsimd) never stalls the next A@V matmuls behind them in
            # the in-order PE queue.
            op_sched = {3: [0, 1, 2, 3], 7: [4, 5, 6, 7],
                        11: [8, 9, 10, 11], 15: [12, 13, 14, 15]}

            def do_consume():
                blk, hp, skt, pt, q0 = queue.pop(0)
                while 4 * (blk + 1) > v_emitted[0]:
                    proj_v_block(v_emitted[0] // 4)
                ctxps = ctx_maps.setdefault((blk, hp), {})
                _consume(st, blk, hp, skt, pt, q0, ctxps)
                consumed[0] += 1
                if op_ready and op_ready[0][1] <= consumed[0]:
                    _outproj_tile(st, op_ready.pop(0)[0])
                if skt == 4 * (blk + 1) - 1:  # pair complete
                    tail = (blk == NB - 1 and hp == 1)
                    for hh2 in range(2):
                        _normalize(st, blk, hp, hh2, ctxps, tail)
                        norms_done[0] += 1
                        for t in op_sched.get(norms_done[0] - 1, []):
                            op_ready.append((t, consumed[0] + 2))
                    del ctx_maps[(blk, hp)]

            # ---- the weave ----
            total = sum(4 * (b + 1) for b in range(NB)) * 2
            emitted = 0
            for blk in range(NB):
                for hp in range(2):
                    ensure_proj("q", blk, hp)
                    for skt in range(4 * (blk + 1)):
                        ensure_proj("k", skt // 4, hp)
                        produce_step(blk, hp, skt)
                        emitted += 1
                        # early V-projection filler keeps the PE dense in
                        # the trimmed-diag stretches of blocks 1-2
                        if (blk, hp, skt) == (1, 1, 3) and v_emitted[0] == 8:
                            proj_v_block(2)
                        if (blk, hp, skt) == (2, 0, 5) and v_emitted[0] == 12:
                            proj_v_block(3)
                        while len(queue) > min(8, total - emitted):
                            do_consume()
            while queue:
                do_consume()
            while op_ready:
                _outproj_tile(st, op_ready.pop(0)[0])


def _produce(st, blk, hp, skt):
    """Scores matmuls + exp (+ causal triangle mask) for one task.

    The two heads of the pair sit on disjoint PE row groups (lhsT base
    partitions 0 and 64), so their back-to-back scores matmuls execute
    concurrently in the array; both heads' P^T share one [128,1024] tile
    (head h2 in columns 512*h2..512*h2+512) and one exp ACTIVATE.
    Diagonal tasks (skt >= 4*blk) only touch query columns >= 128*u."""
    nc = st["nc"]
    u = skt - 4 * blk
    q0 = 128 * u if u > 0 else 0
    sc = st["sc_pool"].tile([128, 1024], F32, name="sc", tag="sc")
    for h2 in range(2):
        nc.tensor.matmul(
            sc[:, ds(512 * h2 + q0, 512 - q0)],
            lhsT=st["kt"][ds(64 * h2, 64), hp, ds(128 * skt, 128)],
            rhs=st["qt"][ds(64 * h2, 64), hp, ds(BLK * blk + q0, BLK - q0)],
            start=True, stop=True)
    pt = st["pt_pool"].tile([128, 1024], DT, name="pt", tag="pt")
    sc3 = sc[:].rearrange("p (h q) -> p h q", h=2)[:, :, q0:BLK]
    pt3 = pt[:].rearrange("p (h q) -> p h q", h=2)[:, :, q0:BLK]
    nc.scalar.activation(pt3, sc3, st["EXP"], scale=0.125)
    if u >= 0:  # diagonal: zero the in-tile upper triangle (both heads)
        tri = pt[:].rearrange("p (h q) -> p h q", h=2)[:, :, q0:q0 + 128]
        nc.vector.tensor_tensor(tri, tri, st["mask"][:], st["MUL"])
    return pt, q0


def _consume(st, blk, hp, skt, pt, q0, ctxps):
    """A@V accumulation for one produced task (both heads of the pair)."""
    nc = st["nc"]
    last = 4 * (blk + 1) - 1
    for h2 in range(2):
        h = 2 * hp + h2
        if h not in ctxps:
            ctxps[h] = st["ctx_pool"].tile(
                [128, BLK], F32, name=f"ctx{h2}", tag="ctx")
        nc.tensor.matmul(
            ctxps[h][0:DK + 1, ds(q0, BLK - q0)],
            lhsT=st["v"][:, skt, ds(h * VW, VW)],
            rhs=pt[:, ds(512 * h2 + q0, BLK - q0)],
            start=(skt == 0), stop=(skt == last))


def _normalize(st, blk, hp, h2, ctxps, tail=False):
    """ctx rows 0..63 scaled by 1/row64 -> ctx^T bf16.

    Normally the ctx PSUM slot is released by two quick DVE copies (sums
    row + ctx rows into SBUF) so the reciprocal/broadcast/multiply run off
    the critical path and the next pair's A@V is not stalled.  For the
    last pair (tail=True) the multiply reads the ctx PSUM directly
    (nothing follows, and skipping the raw copy shortens the final
    normalize->outproj chain), and a result-unused PE outer product keeps
    the HAM clock-gate warm through the drain."""
    nc = st["nc"]
    h = 2 * hp + h2
    # custom-DVE ops read garbage from PSUM -> plain-copy the sums row to
    # SBUF first (DVE copy of [1,512] is cheap; DVE reads PSUM fine).
    sums = st["small"].tile([1, BLK], F32, name="sums", tag="sums")
    nc.vector.tensor_copy(sums[:], ctxps[h][ds(DK, 1), :])
    if tail:
        raw = ctxps[h][0:64, :]
    else:
        raw_t = st["small"].tile([64, BLK], F32, name="raw", tag="raw")
        nc.vector.tensor_copy(raw_t[:], ctxps[h][0:64, :])
        raw = raw_t[:]
    r = st["small"].tile([1, BLK], F32, name="r", tag="r")
    nc.vector.reciprocal_approx_fast(out=r[:], in_=sums[:])
    if tail:  # HAM warm-keeper (result never read; slot frees on write)
        wm = st["sc_pool"].tile([128, 1024], F32, name="sc", tag="sc")
        nc.tensor.matmul(wm[0:DK, 0:BLK], lhsT=st["ones"][:], rhs=r[:],
                         start=True, stop=True)
    bc = st["small"].tile([64, BLK], F32, name="bc", tag="bc")
    nc.gpsimd.partition_broadcast(bc[:], r[:])
    nc.vector.tensor_tensor(
        st["ctxt"][ds(64 * h2, 64), hp, ds(BLK * blk, BLK)],
        raw, bc[:], st["MUL"])


def _outproj_tile(st, t):
    nc = st["nc"]
    ob = st["out_pool"].tile([128, D], DT, name="ob", tag="ob")
    pp = st["sc_pool"].tile([128, 1024], F32, name="sc", tag="sc")
    for nb in range(2):
        for cc in range(2):
            nc.tensor.matmul(
                pp[:, ds(512 * nb, 512)],
                lhsT=st["ctxt"][:, cc, ds(128 * t, 128)],
                rhs=st["wo"][:, cc, ds(512 * nb, 512)],
                start=(cc == 0), stop=(cc == 1))
    nc.vector.tensor_copy(ob[:], pp[:])
    nc.sync.dma_start(st["out_d"][ds(128 * t, 128), :], ob[:])


def _make_mask():
    # tri[i, h, j] = 1.0 iff key-within-tile i <= query-within-group j,
    # duplicated for the two heads of a pair (shared P^T tile).
    i = np.arange(128)[:, None]
    j = np.arange(128)[None, :]
    tri = (i <= j).astype(NP_DT)
    return np.ascontiguousarray(np.stack([tri, tri], axis=1))


def _prep_core_inputs(inputs, core):
    b = core // 4
    h0 = HPC * (core % 4)
    c0, c1 = h0 * DK, (h0 + HPC) * DK
    f32 = np.float32

    def t_blocks(x):  # [S, D] -> [128, NB, NCH, BLK]
        xt = np.ascontiguousarray(np.asarray(x, f32).T)  # [D, S]
        return np.ascontiguousarray(
            xt.reshape(NCH, 128, NB, BLK).transpose(1, 2, 0, 3)
        ).astype(NP_DT)

    return {
        "xq_t": t_blocks(inputs["input_Q"][b]),
        "xk_t": t_blocks(inputs["input_K"][b]),
        "xv_t": t_blocks(inputs["input_V"][b]),
        "wq": np.ascontiguousarray(np.asarray(inputs["W_Q"], f32)[:, c0:c1].reshape(NCH, 128, 2, 128).transpose(1, 2, 0, 3)).astype(NP_DT),
        "wk": np.ascontiguousarray(np.asarray(inputs["W_K"], f32)[:, c0:c1].reshape(NCH, 128, 2, 128).transpose(1, 2, 0, 3)).astype(NP_DT),
        "wv": np.ascontiguousarray(np.asarray(inputs["W_V"], f32)[:, c0:c1].reshape(NCH, 128, HPC * DK).transpose(1, 0, 2)).astype(NP_DT),
        "wo": np.ascontiguousarray(np.asarray(inputs["W_O"], f32)[c0:c1, :].reshape(2, 128, D).transpose(1, 0, 2)).astype(NP_DT),
        "mask01": _make_mask(),
        "ident": np.eye(128, dtype=f32),
    }


def get_program():
    global _CACHED_NC
    if _CACHED_NC is None:
        _CACHED_NC = _build_program()
    return _CACHED_NC


def kernel(**inputs):
    global LAST_RESULTS
    nc = get_program()
    in_maps = [_prep_core_inputs(inputs, core) for core in range(N_CORES)]
    res = bass_utils.run_bass_kernel_spmd(
        nc, in_maps, core_ids=list(range(N_CORES)),
        trace=TRACE or bool(int(os.environ.get("BASS_TRACE", "0") or 0)))
    LAST_RESULTS = res
    out = np.zeros((B, S, D), np.float32)
    for core in range(N_CORES):
        out[core // 4] += np.asarray(
            res.results[core]["out_partial"], dtype=np.float32)
    return out


# revision 36
# speedup vs baseline: 1.0855x; 1.0186x over previous
"""Multi-head attention (B=2, S=2048, D=1024, H=16, dk=dv=64) on 8 trn2 cores.

Sharding: (batch, head-quad) -> core.  Core i handles batch i//4 and the 4
heads [4*(i%4), 4*(i%4)+4).  Each core computes its partial output
context_h @ W_O[h-slice] summed over its 4 heads; the host sums the 4
bf16 partials per batch in f32 (the "all-reduce" of the row-sharded output
projection).

The main loop is ACT(exp)-bound: 80 score tasks x ~1us of exp.  The
schedule therefore starts the exp stream as early as possible (first
score task after ~16 matmuls) and keeps ACT saturated: projection halves
are emitted lazily right before the first score task that needs them,
consumes trail produces by a bounded backlog, and causal trimming cuts
every diagonal task (scores matmul, exp ACTIVATE via a 3-D AP over both
heads' valid ranges, A@V matmul, and a 128-wide triangle-only mask).

PSUM discipline (in-order engine queues make ring-allocation order
load-bearing): one uniform transient ring of 2x[128,1024] (tag "sc")
carries ALL matmul outputs that are freed immediately by ACT or a DVE
copy -- scores, Q/K projection halves, V projection, output projection.
The 4x[128,512] ctx ring holds ONLY the A@V pair accumulators (two pairs
in flight), so no transient allocation can ever ring-wait on a held ctx
slot (deadlock-free by construction).

Inputs are host-packed as [128, NB, NCH, BLK] so each 512-query block of
x^T is a single DMA descriptor (descriptor issue on the Sync queue costs
~0.7us each); xv streams through a 2-deep [128, NCH, BLK] rotating pool.
"""

import os
import numpy as np
import ml_dtypes

import concourse.bacc as bacc
import concourse.tile as tile
import concourse.mybir as mybir
import concourse.bass_utils as bass_utils
from concourse.bass import ds

B, S, D, H, DK = 2, 2048, 1024, 16, 64
N_CORES = 8
HPC = 4            # heads per core
NCH = 8            # d-model chunks of 128
NB = 4             # query blocks of 512
BLK = 512
NT = 16            # s tiles of 128
VW = DK + 1        # V columns per head incl. ones column

DT = mybir.dt.bfloat16
NP_DT = ml_dtypes.bfloat16
F32 = mybir.dt.float32

TRACE = False      # set True (or BASS_TRACE=1) to capture an NTFF profile
LAST_RESULTS = None

_CACHED_NC = None


def _build_program():
    nc = bacc.Bacc("TRN2", target_bir_lowering=False, debug=False,
                   enable_asserts=False, num_devices=N_CORES)

    xq_d = nc.dram_tensor("xq_t", [128, NB, NCH, BLK], DT, kind="ExternalInput")
    xk_d = nc.dram_tensor("xk_t", [128, NB, NCH, BLK], DT, kind="ExternalInput")
    xv_d = nc.dram_tensor("xv_t", [128, NB, NCH, BLK], DT, kind="ExternalInput")
    wq_d = nc.dram_tensor("wq", [128, 2, NCH, 128], DT, kind="ExternalInput")
    wk_d = nc.dram_tensor("wk", [128, 2, NCH, 128], DT, kind="ExternalInput")
    wv_d = nc.dram_tensor("wv", [128, NCH, HPC * DK], DT, kind="ExternalInput")
    wo_d = nc.dram_tensor("wo", [128, 2, D], DT, kind="ExternalInput")
    mask_d = nc.dram_tensor("mask01", [128, 2, 128], DT, kind="ExternalInput")
    ident_d = nc.dram_tensor("ident", [128, 128], F32, kind="ExternalInput")
    out_d = nc.dram_tensor("out_partial", [S, D], DT, kind="ExternalOutput")

    with tile.TileContext(nc) as tc:
        _body(tc, xq_d, xk_d, xv_d, wq_d, wk_d, wv_d, wo_d, mask_d, ident_d,
              out_d)
    nc.compile()
    return nc


def _body(tc, xq_d, xk_d, xv_d, wq_d, wk_d, wv_d, wo_d, mask_d, ident_d,
          out_d):
    nc = tc.nc

    with (
        tc.tile_pool(name="consts", bufs=1) as consts,
        tc.tile_pool(name="persist", bufs=1) as persist,
        tc.tile_pool(name="small", bufs=3) as small,
    ):
        # ---- constants ----
        wq_sb = consts.tile([128, 2, NCH, 128], DT)
        wk_sb = consts.tile([128, 2, NCH, 128], DT)
        wv_sb = consts.tile([128, NCH, HPC * DK], DT)
        wo_sb = consts.tile([128, 2, D], DT)
        mask_sb = consts.tile([128, 2, 128], DT)
        ident_sb = consts.tile([128, 128], F32)

        # ---- persistent activations ----
        qt_sb = persist.tile([128, 2, S], DT)        # Q^T, pair-major
        kt_sb = persist.tile([128, 2, S], DT)        # K^T
        v_sb = persist.tile([128, NT, HPC * VW], DT)  # V + ones cols
        ctxt_sb = persist.tile([128, 2, S], DT)      # context^T

        ones_sb = persist.tile([1, DK], F32)    # for the 1/denom broadcast
        nc.vector.memset(ones_sb[:], 1.0)
        for hh in range(HPC):
            nc.vector.memset(v_sb[:, :, hh * VW + DK: hh * VW + DK + 1], 1.0)

        with (
            tc.tile_pool(name="xqk", bufs=1) as xqk_pool,
            tc.tile_pool(name="xv", bufs=3) as xv_pool,
            tc.tile_pool(name="pt", bufs=18) as pt_pool,
            tc.tile_pool(name="osb", bufs=3) as out_pool,
            tc.tile_pool(name="psum_sc", bufs=3, space="PSUM") as sc_pool,
            tc.tile_pool(name="psum_ctx", bufs=2, space="PSUM") as ctx_pool,
        ):
            st = dict(sc_pool=sc_pool, ctx_pool=ctx_pool,
                      pt_pool=pt_pool, out_pool=out_pool, small=small,
                      qt=qt_sb, kt=kt_sb, v=v_sb, ctxt=ctxt_sb,
                      mask=mask_sb, wo=wo_sb, out_d=out_d, nc=nc,
                      ones=ones_sb,
                      EXP=mybir.ActivationFunctionType.Exp,
                      MUL=mybir.AluOpType.mult)
            xq_sb = xqk_pool.tile([128, NB, NCH, BLK], DT)
            xk_sb = xqk_pool.tile([128, NB, NCH, BLK], DT)

            # ---- DMA issue order = need order ----
            nc.sync.dma_start(wq_sb[:, 0], wq_d[:, 0])
            for c4 in range(0, NCH, 4):
                nc.sync.dma_start(xq_sb[:, 0, c4:c4 + 4],
                                  xq_d[:, 0, c4:c4 + 4])
            nc.sync.dma_start(wk_sb[:, 0], wk_d[:, 0])
            for c4 in range(0, NCH, 4):
                nc.sync.dma_start(xk_sb[:, 0, c4:c4 + 4],
                                  xk_d[:, 0, c4:c4 + 4])
            nc.sync.dma_start(wq_sb[:, 1], wq_d[:, 1])
            nc.sync.dma_start(wk_sb[:, 1], wk_d[:, 1])
            nc.sync.dma_start(mask_sb[:], mask_d[:])
            nc.sync.dma_start(ident_sb[:], ident_d[:])
            xv_bufs = {}

            def xv_fetch(b):
                t = xv_pool.tile([128, NCH, BLK], DT, name="xvb", tag="xvb")
                nc.sync.dma_start(t[:], xv_d[:, b])
                xv_bufs[b] = t

            # per-block input slices in need order: block 1 feeds the
            # (1,*) projection halves long before blocks 2/3 are touched
            nc.sync.dma_start(xq_sb[:, 1], xq_d[:, 1])
            nc.sync.dma_start(xk_sb[:, 1], xk_d[:, 1])
            xv_fetch(0)
            nc.sync.dma_start(wv_sb[:], wv_d[:])
            nc.sync.dma_start(xq_sb[:, 2], xq_d[:, 2])
            nc.sync.dma_start(xk_sb[:, 2], xk_d[:, 2])
            nc.sync.dma_start(xq_sb[:, 3], xq_d[:, 3])
            nc.sync.dma_start(xk_sb[:, 3], xk_d[:, 3])
            nc.sync.dma_start(wo_sb[:], wo_d[:])

            # ---- weave state ----
            queue = []      # produced-but-unconsumed (blk, hp, skt, pt, q0)
            ctx_maps = {}   # (blk, hp) -> {h: psum tile}
            norms_done = [0]
            v_emitted = [0]
            consumed = [0]
            op_ready = []   # (tile, min_consumed)
            proj_emitted = set()

            def transient():
                return sc_pool.tile([128, 1024], F32, name="sc", tag="sc")

            def proj_half(dst, w_sb, blk0, nblk, p):
                x_sb = xq_sb if dst is qt_sb else xk_sb
                w = BLK * nblk
                ps = transient()
                for c in range(NCH):
                    nc.tensor.matmul(
                        ps[:, 0:w],
                        lhsT=w_sb[:, p, c, :],
                        rhs=x_sb[:, blk0:blk0 + nblk, c, :],
                        start=(c == 0), stop=(c == NCH - 1))
                nc.vector.tensor_copy(dst[:, p, ds(BLK * blk0, w)],
                                      ps[:, 0:w])

            # projection emission groups (one matmul output <= one PSUM
            # bank = 512 fp32, so groups stay single-block)
            PGRP = {0: (0, 1), 1: (1, 1), 2: (2, 1), 3: (3, 1)}

            def ensure_proj(which, blk, p):
                g = PGRP[blk]
                if (which, g, p) in proj_emitted:
                    return
                proj_emitted.add((which, g, p))
                if which == "q":
                    proj_half(qt_sb, wq_sb, g[0], g[1], p)
                else:
                    proj_half(kt_sb, wk_sb, g[0], g[1], p)

            def proj_v_block(b):
                for nb in range(b + 1, min(NB, b + 3)):
                    if nb not in xv_bufs:
                        xv_fetch(nb)
                x = xv_bufs.pop(b)
                for t in range(4 * b, 4 * b + 4):
                    ps = transient()
                    for c in range(NCH):
                        nc.tensor.matmul(
                            ps[:, 0:HPC * DK],
                            lhsT=x[:, c, ds(128 * (t - 4 * b), 128)],
                            rhs=wv_sb[:, c, :],
                            start=(c == 0), stop=(c == NCH - 1))
                    dst = v_sb[:, t, :].rearrange(
                        "p (hh e) -> p hh e", hh=HPC)[:, :, 0:DK]
                    srcap = ps[:, 0:HPC * DK].rearrange(
                        "p (hh e) -> p hh e", hh=HPC)
                    nc.vector.tensor_copy(dst, srcap)
                v_emitted[0] = 4 * b + 4

            def produce_step(blk, hp, skt):
                pt, q0 = _produce(st, blk, hp, skt)
                queue.append((blk, hp, skt, pt, q0))

            # outproj tiles released per normalize event; emitted one per
            # consume, >=2 consumes after the event, so the normalize chain
            # (DVE/gpsimd) never stalls the next A@V matmuls behind them in
            # the in-order PE queue.
            op_sched = {3: [0, 1, 2, 3], 7: [4, 5, 6, 7],
                        11: [8, 9, 10, 11], 15: [12, 13, 14, 15]}

            def do_consume():
                blk, hp, skt, pt, q0 = queue.pop(0)
                while 4 * (blk + 1) > v_emitted[0]:
                    proj_v_block(v_emitted[0] // 4)
                ctxps = ctx_maps.setdefault((blk, hp), {})
                _consume(st, blk, hp, skt, pt, q0, ctxps)
                consumed[0] += 1
                if op_ready and op_ready[0][1] <= consumed[0]:
                    _outproj_tile(st, op_ready.pop(0)[0])
                if skt == 4 * (blk + 1) - 1:  # pair complete
                    tail = (blk == NB - 1 and hp == 1)
                    for hh2 in range(2):
                        _normalize(st, blk, hp, hh2, ctxps, tail)
                        norms_done[0] += 1
                        for t in op_sched.get(norms_done[0] - 1, []):
                            op_ready.append((t, consumed[0] + 2))
                    del ctx_maps[(blk, hp)]

            # ---- the weave ----
            total = sum(4 * (b + 1) for b in range(NB)) * 2
            emitted = 0
            for blk in range(NB):
                for hp in range(2):
                    ensure_proj("q", blk, hp)
                    for skt in range(4 * (blk + 1)):
                        ensure_proj("k", skt // 4, hp)
                        produce_step(blk, hp, skt)
                        emitted += 1
                        # early V-projection filler keeps the PE dense in
                        # the trimmed-diag stretches of blocks 1-2
                        if (blk, hp, skt) == (1, 1, 3) and v_emitted[0] == 8:
                            proj_v_block(2)
                        if (blk, hp, skt) == (2, 0, 5) and v_emitted[0] == 12:
                            proj_v_block(3)
                        while len(queue) > min(10, total - emitted):
                            do_consume()
            while queue:
                do_consume()
            while op_ready:
                _outproj_tile(st, op_ready.pop(0)[0])


def _produce(st, blk, hp, skt):
    """Scores matmuls + exp (+ causal triangle mask) for one task.

    The two heads of the pair sit on disjoint PE row groups (lhsT base
    partitions 0 and 64), so their back-to-back scores matmuls execute
    concurrently in the array; both heads' P^T share one [128,1024] tile
    (head h2 in columns 512*h2..512*h2+512) and one exp ACTIVATE.
    Diagonal tasks (skt >= 4*blk) only touch query columns >= 128*u."""
    nc = st["nc"]
    u = skt - 4 * blk
    q0 = 128 * u if u > 0 else 0
    sc = st["sc_pool"].tile([128, 1024], F32, name="sc", tag="sc")
    for h2 in range(2):
        nc.tensor.matmul(
            sc[:, ds(512 * h2 + q0, 512 - q0)],
            lhsT=st["kt"][ds(64 * h2, 64), hp, ds(128 * skt, 128)],
            rhs=st["qt"][ds(64 * h2, 64), hp, ds(BLK * blk + q0, BLK - q0)],
            start=True, stop=True)
    pt = st["pt_pool"].tile([128, 1024], DT, name="pt", tag="pt")
    sc3 = sc[:].rearrange("p (h q) -> p h q", h=2)[:, :, q0:BLK]
    pt3 = pt[:].rearrange("p (h q) -> p h q", h=2)[:, :, q0:BLK]
    nc.scalar.activation(pt3, sc3, st["EXP"], scale=0.125)
    if u >= 0:  # diagonal: zero the in-tile upper triangle (both heads)
        tri = pt[:].rearrange("p (h q) -> p h q", h=2)[:, :, q0:q0 + 128]
        nc.vector.tensor_tensor(tri, tri, st["mask"][:], st["MUL"])
    return pt, q0


def _consume(st, blk, hp, skt, pt, q0, ctxps):
    """A@V accumulation for one produced task (both heads of the pair)."""
    nc = st["nc"]
    last = 4 * (blk + 1) - 1
    for h2 in range(2):
        h = 2 * hp + h2
        if h not in ctxps:
            ctxps[h] = st["ctx_pool"].tile(
                [128, BLK], F32, name=f"ctx{h2}", tag="ctx")
        nc.tensor.matmul(
            ctxps[h][0:DK + 1, ds(q0, BLK - q0)],
            lhsT=st["v"][:, skt, ds(h * VW, VW)],
            rhs=pt[:, ds(512 * h2 + q0, BLK - q0)],
            start=(skt == 0), stop=(skt == last))


def _normalize(st, blk, hp, h2, ctxps, tail=False):
    """ctx rows 0..63 scaled by 1/row64 -> ctx^T bf16.

    Normally the ctx PSUM slot is released by two quick DVE copies (sums
    row + ctx rows into SBUF) so the reciprocal/broadcast/multiply run off
    the critical path and the next pair's A@V is not stalled.  For the
    last pair (tail=True) the multiply reads the ctx PSUM directly
    (nothing follows, and skipping the raw copy shortens the final
    normalize->outproj chain), and a result-unused PE outer product keeps
    the HAM clock-gate warm through the drain."""
    nc = st["nc"]
    h = 2 * hp + h2
    # custom-DVE ops read garbage from PSUM -> plain-copy the sums row to
    # SBUF first (DVE copy of [1,512] is cheap; DVE reads PSUM fine).
    sums = st["small"].tile([1, BLK], F32, name="sums", tag="sums")
    nc.vector.tensor_copy(sums[:], ctxps[h][ds(DK, 1), :])
    if tail:
        raw = ctxps[h][0:64, :]
    else:
        raw_t = st["small"].tile([64, BLK], F32, name="raw", tag="raw")
        nc.vector.tensor_copy(raw_t[:], ctxps[h][0:64, :])
        raw = raw_t[:]
    r = st["small"].tile([1, BLK], F32, name="r", tag="r")
    nc.vector.reciprocal_approx_fast(out=r[:], in_=sums[:])
    if tail:  # HAM warm-keeper (result never read; slot frees on write)
        wm = st["sc_pool"].tile([128, 1024], F32, name="sc", tag="sc")
        nc.tensor.matmul(wm[0:DK, 0:BLK], lhsT=st["ones"][:], rhs=r[:],
                         start=True, stop=True)
    bc = st["small"].tile([64, BLK], F32, name="bc", tag="bc")
    nc.gpsimd.partition_broadcast(bc[:], r[:])
    nc.vector.tensor_tensor(
        st["ctxt"][ds(64 * h2, 64), hp, ds(BLK * blk, BLK)],
        raw, bc[:], st["MUL"])


def _outproj_tile(st, t):
    nc = st["nc"]
    ob = st["out_pool"].tile([128, D], DT, name="ob", tag="ob")
    pp = st["sc_pool"].tile([128, 1024], F32, name="sc", tag="sc")
    for nb in range(2):
        for cc in range(2):
            nc.tensor.matmul(
                pp[:, ds(512 * nb, 512)],
                lhsT=st["ctxt"][:, cc, ds(128 * t, 128)],
                rhs=st["wo"][:, cc, ds(512 * nb, 512)],
                start=(cc == 0), stop=(cc == 1))
    nc.vector.tensor_copy(ob[:], pp[:])
    nc.sync.dma_start(st["out_d"][ds(128 * t, 128), :], ob[:])


def _make_mask():
    # tri[i, h, j] = 1.0 iff key-within-tile i <= query-within-group j,
    # duplicated for the two heads of a pair (shared P^T tile).
    i = np.arange(128)[:, None]
    j = np.arange(128)[None, :]
    tri = (i <= j).astype(NP_DT)
    return np.ascontiguousarray(np.stack([tri, tri], axis=1))


def _prep_core_inputs(inputs, core):
    b = core // 4
    h0 = HPC * (core % 4)
    c0, c1 = h0 * DK, (h0 + HPC) * DK
    f32 = np.float32

    def t_blocks(x):  # [S, D] -> [128, NB, NCH, BLK]
        xt = np.ascontiguousarray(np.asarray(x, f32).T)  # [D, S]
        return np.ascontiguousarray(
            xt.reshape(NCH, 128, NB, BLK).transpose(1, 2, 0, 3)
        ).astype(NP_DT)

    return {
        "xq_t": t_blocks(inputs["input_Q"][b]),
        "xk_t": t_blocks(inputs["input_K"][b]),
        "xv_t": t_blocks(inputs["input_V"][b]),
        "wq": np.ascontiguousarray(np.asarray(inputs["W_Q"], f32)[:, c0:c1].reshape(NCH, 128, 2, 128).transpose(1, 2, 0, 3)).astype(NP_DT),
        "wk": np.ascontiguousarray(np.asarray(inputs["W_K"], f32)[:, c0:c1].reshape(NCH, 128, 2, 128).transpose(1, 2, 0, 3)).astype(NP_DT),
        "wv": np.ascontiguousarray(np.asarray(inputs["W_V"], f32)[:, c0:c1].reshape(NCH, 128, HPC * DK).transpose(1, 0, 2)).astype(NP_DT),
        "wo": np.ascontiguousarray(np.asarray(inputs["W_O"], f32)[c0:c1, :].reshape(2, 128, D).transpose(1, 0, 2)).astype(NP_DT),
        "mask01": _make_mask(),
        "ident": np.eye(128, dtype=f32),
    }


def get_program():
    global _CACHED_NC
    if _CACHED_NC is None:
        _CACHED_NC = _build_program()
    return _CACHED_NC


def kernel(**inputs):
    global LAST_RESULTS
    nc = get_program()
    in_maps = [_prep_core_inputs(inputs, core) for core in range(N_CORES)]
    res = bass_utils.run_bass_kernel_spmd(
        nc, in_maps, core_ids=list(range(N_CORES)),
        trace=TRACE or bool(int(os.environ.get("BASS_TRACE", "0") or 0)))
    LAST_RESULTS = res
    out = np.zeros((B, S, D), np.float32)
    for core in range(N_CORES):
        out[core // 4] += np.asarray(
            res.results[core]["out_partial"], dtype=np.float32)
    return out
